# revision 9
# baseline (speedup 1.0000x reference)
"""CGC multi-task MoE kernel for Trainium2 (8 NeuronCores, data-parallel over batch).

Model (per token): 16 unique expert MLPs 256->128(relu)->64 (12 task-specific +
4 shared), 3 task gates softmax(x@gw[t]) over 8 experts each, outputs are the
gate-weighted sums. out[t] = sum_e g[t,:,e] * expert_e(x).

v3 layout strategy (per core, Bc=8192 tokens, 16 tiles of 512):
 - x is host-quantized to DUAL-LEVEL fp8 e4m3 (x8 + xr8, xr8 = q8(x - x8)) and
   shipped feature-major as DoubleRow planes [128, 2, Bc] (k-slice on planes).
 - L1 per expert: 3 fp8 DoubleRow matmuls (w18*x8 + w18*xr8 + w1r8*x8; the
   w1r8*xr8 term ~eps^2 is dropped). w1 is scaled by 32 before quantization so
   the residual level stays clear of e4m3's subnormal range; the 1/32 descale
   rides on the L2-output evacuation. DoubleRow = 0.5 cyc/row with 2 K-planes
   per MM -> 4x f32r L1 throughput.
 - experts processed as PAIRS: both experts' h land in one [128,2,512] PSUM
   group (2 banks), one batched relu evacuates them to SBUF f32r.
 - L2: f32r masked-pair matmuls; two pairs' o in one [128,2,512] PSUM group,
   one batched evac (scale 1/32) to SBUF.
 - gates: logits via 3 fp8 DoubleRow MMs (dual x (x) dual 32*gw), exp with
   scale=1/32 bias=gb on ScalarE, sums/16-over-sum via K=128 zero-padded f32r
   MMs, gnorm*16 split to dual fp8 (hi/lo DoubleRow planes).
 - gate broadcast: per (task, q) one fp8 DoubleRow MM (indicator stationary,
   entries 1/16) over the gnorm hi/lo planes; the two q's of a (task, group)
   land in one [128,2,512] PSUM group -> ONE DVE multiply per (task, group)
   (6 per tile) produces bf16 gated tiles.
 - combine: 3 bf16 SBUF adds per task (DVE/GpSimd), final cross-partition
   128->64 pair-sum via one tiny PE matmul per task (shared 0/1 stationary);
   tasks 0/1 share a PSUM bank via tile_position col 0/64.
 - output bf16 [192, Bc], converted to fp32 on host.

Fast path assumes b1 = b2 = 0 (guaranteed by the problem spec: bias fills are
zeros); a slower exact-bias variant is compiled on demand if any bias is
nonzero. The gate bias gb is always handled exactly (free via exp bias).
"""

import sys

if "/opt/trn_rl_repo" not in sys.path:
    sys.path.insert(0, "/opt/trn_rl_repo")

import numpy as np
import ml_dtypes
from contextlib import ExitStack

import concourse.bass as bass
import concourse.bacc as bacc
import concourse.tile as tile
from concourse import mybir
from concourse.bass_utils import run_bass_kernel_spmd

B, D, H, O = 65536, 256, 128, 64
NS, NSH, NT = 4, 4, 3
NE = NS + NSH            # 8 experts per task's gate
NEXP = NT * NS + NSH     # 16 unique experts
NCORES = 8
BC = B // NCORES         # 8192 tokens per core
BT = 512                 # tokens per tile
NTILES = BC // BT        # 16

f32 = mybir.dt.float32
f32r = mybir.dt.float32r
bf16 = mybir.dt.bfloat16
f8 = mybir.dt.float8e4
F8NP = ml_dtypes.float8_e4m3

WS = 32.0                # weight pre-quantization scale (w1, gw)
GS = 16.0                # gnorm scale

# L2 pairs: global expert ids (0..11 task-specific, 12..15 shared)
L2_PAIRS = [(2 * p, 2 * p + 1) for p in range(8)]
AluOp = mybir.AluOpType
DR = mybir.MatmulPerfMode.DoubleRow


def _build_nc(with_bias: bool):
    nc = bacc.Bacc("TRN2", target_bir_lowering=False, debug=False, num_devices=NCORES)
    dram = {}
    dram["X8"] = nc.dram_tensor("X8", [128, 2, BC], f8, kind="ExternalInput").ap()
    dram["XR8"] = nc.dram_tensor("XR8", [128, 2, BC], f8, kind="ExternalInput").ap()
    dram["W18"] = nc.dram_tensor("W18", [128, NEXP, 2, 128], f8, kind="ExternalInput").ap()
    dram["W1R8"] = nc.dram_tensor("W1R8", [128, NEXP, 2, 128], f8, kind="ExternalInput").ap()
    dram["W2"] = nc.dram_tensor("W2", [128, NEXP * 128], f32r, kind="ExternalInput").ap()
    dram["GW8"] = nc.dram_tensor("GW8", [128, 2, 32], f8, kind="ExternalInput").ap()
    dram["GWR8"] = nc.dram_tensor("GWR8", [128, 2, 32], f8, kind="ExternalInput").ap()
    dram["IND8"] = nc.dram_tensor("IND8", [128, NT * NS, 2, 128], f8, kind="ExternalInput").ap()
    dram["FOLDH"] = nc.dram_tensor("FOLDH", [128, 64], bf16, kind="ExternalInput").ap()
    dram["E"] = nc.dram_tensor("E", [128, NT], f32r, kind="ExternalInput").ap()
    dram["R16"] = nc.dram_tensor("R16", [128, NT * NE], f32r, kind="ExternalInput").ap()
    dram["B1S"] = nc.dram_tensor("B1S", [128, NEXP], f32, kind="ExternalInput").ap()
    dram["B2"] = nc.dram_tensor("B2", [128, 8], f32, kind="ExternalInput").ap()
    dram["GB"] = nc.dram_tensor("GB", [NT * NE, 1], f32, kind="ExternalInput").ap()
    dram["ZPAD"] = nc.dram_tensor("ZPAD", [128, BT], f32r, kind="ExternalInput").ap()
    dram["ZPAD8"] = nc.dram_tensor("ZPAD8", [128, 2, BT], f8, kind="ExternalInput").ap()
    out_dram = nc.dram_tensor("out", [NT * O, BC], bf16, kind="ExternalOutput").ap()

    AF = mybir.ActivationFunctionType

    with tile.TileContext(nc) as tc:
        with ExitStack() as ctx:
            const = ctx.enter_context(tc.tile_pool(name="const", bufs=1))
            xpool = ctx.enter_context(tc.tile_pool(name="x", bufs=6))
            sbH = ctx.enter_context(tc.tile_pool(name="sbH", bufs=3))
            sbO = ctx.enter_context(tc.tile_pool(name="sbO", bufs=6))
            sbG = ctx.enter_context(tc.tile_pool(name="sbG", bufs=9))
            sbS = ctx.enter_context(tc.tile_pool(name="sbS", bufs=3))
            sbOut = ctx.enter_context(tc.tile_pool(name="sbOut", bufs=4))
            psH = ctx.enter_context(tc.tile_pool(name="psH", bufs=1, space="PSUM"))
            psO = ctx.enter_context(tc.tile_pool(name="psO", bufs=1, space="PSUM"))
            psB = ctx.enter_context(tc.tile_pool(name="psB", bufs=1, space="PSUM"))
            psF = ctx.enter_context(tc.tile_pool(name="psF", bufs=1, space="PSUM"))

            # static K-padded buffers (zero rows so K=128 matmuls see exact
            # zeros). expg: rows 24:128; recip: rows 3:128; gnorm8: rows
            # 24:128 on both planes.
            expg_bufs, recip_bufs, gn8_bufs = [], [], []
            for nb in range(2):
                eb = nc.alloc_sbuf_tensor(f"expgP{nb}", [128, BT], f32r).ap()
                rb = nc.alloc_sbuf_tensor(f"recipP{nb}", [128, BT], f32r).ap()
                gb8 = nc.alloc_sbuf_tensor(f"gn8P{nb}", [128, 2, BT], f8).ap()
                nc.sync.dma_start(eb[24:128, :], dram["ZPAD"][24:128, :])
                nc.sync.dma_start(rb[3:128, :], dram["ZPAD"][3:128, :])
                nc.sync.dma_start(gb8[24:128, :, :], dram["ZPAD8"][24:128, :, :])
                expg_bufs.append(eb)
                recip_bufs.append(rb)
                gn8_bufs.append(gb8)

            x_prefetch = {}

            # ---- load constants (ordered by first use) ----
            GW8sb = const.tile([128, 2, 32], f8, tag="GW8")
            GWR8sb = const.tile([128, 2, 32], f8, tag="GWR8")
            Esb = const.tile([128, NT], f32r, tag="E")
            R16sb = const.tile([128, NT * NE], f32r, tag="R16")
            GBsb = const.tile([NT * NE, 1], f32, tag="GB")
            W18sb = const.tile([128, NEXP, 2, 128], f8, tag="W18")
            W1R8sb = const.tile([128, NEXP, 2, 128], f8, tag="W1R8")
            W2sb = const.tile([128, NEXP * 128], f32r, tag="W2")
            IND8sb = const.tile([128, NT * NS, 2, 128], f8, tag="IND8")
            FOLDHsb = const.tile([128, 64], bf16, tag="FOLDH")
            B1Ssb = const.tile([128, NEXP], f32, tag="B1S")
            B2sb = const.tile([128, 8], f32, tag="B2")
            nc.sync.dma_start(GW8sb[:], dram["GW8"][:])
            nc.sync.dma_start(GWR8sb[:], dram["GWR8"][:])
            nc.sync.dma_start(GBsb[:], dram["GB"][:])
            nc.sync.dma_start(Esb[:], dram["E"][:])
            nc.sync.dma_start(R16sb[:], dram["R16"][:])
            for k in range(2):
                for i0 in range(2):
                    xt = xpool.tile([128, 2, BT], f8, tag=f"x{k}")
                    src = dram["X8"] if k == 0 else dram["XR8"]
                    nc.sync.dma_start(xt[:], src[:, :, bass.ts(i0, BT)])
                    x_prefetch[(i0, k)] = xt
            # shared experts (12..15) lead the pair loop, so their W1 slices go
            # first, then task experts
            nc.sync.dma_start(W18sb[:, 12:16], dram["W18"][:, 12:16])
            nc.sync.dma_start(W1R8sb[:, 12:16], dram["W1R8"][:, 12:16])
            nc.sync.dma_start(W2sb[:, 12 * 128:16 * 128], dram["W2"][:, 12 * 128:16 * 128])
            nc.sync.dma_start(B1Ssb[:], dram["B1S"][:])
            nc.sync.dma_start(B2sb[:], dram["B2"][:])
            for t in range(NT):
                nc.sync.dma_start(W18sb[:, t * NS:(t + 1) * NS], dram["W18"][:, t * NS:(t + 1) * NS])
                nc.sync.dma_start(W1R8sb[:, t * NS:(t + 1) * NS], dram["W1R8"][:, t * NS:(t + 1) * NS])
                nc.sync.dma_start(
                    W2sb[:, t * 4 * 128:(t + 1) * 4 * 128],
                    dram["W2"][:, t * 4 * 128:(t + 1) * 4 * 128],
                )
            nc.sync.dma_start(IND8sb[:], dram["IND8"][:])
            nc.sync.dma_start(FOLDHsb[:], dram["FOLDH"][:])

            for i in range(NTILES):
                # ---- load x tile (fp8 dual planes) ----
                xa = []
                for k in range(2):
                    if (i, k) in x_prefetch:
                        xa.append(x_prefetch.pop((i, k)))
                        continue
                    xt = xpool.tile([128, 2, BT], f8, tag=f"x{k}")
                    src = dram["X8"] if k == 0 else dram["XR8"]
                    nc.sync.dma_start(xt[:], src[:, :, bass.ts(i, BT)])
                    xa.append(xt)
                x8, xr8 = xa

                # ---- gates ----
                glog = psB.tile([32, BT], f32, tag="bc")
                nc.tensor.matmul(glog[:], GW8sb[:], x8[:], start=True, stop=False,
                                 perf_mode=DR)
                nc.tensor.matmul(glog[:], GW8sb[:], xr8[:], start=False, stop=False,
                                 perf_mode=DR)
                nc.tensor.matmul(glog[:], GWR8sb[:], x8[:], start=False, stop=True,
                                 perf_mode=DR)
                expg = expg_bufs[i % 2]
                nc.scalar.activation(expg[0:NT * NE, :], glog[0:NT * NE, :], AF.Exp,
                                     bias=GBsb[:, 0:1], scale=1.0 / WS)
                sums = psB.tile([NT, BT], f32, tag="bc")
                nc.tensor.matmul(sums[:], Esb[:], expg[:], start=True, stop=True)
                recip = recip_bufs[i % 2]
                from concourse.dve_ops import (
                    RECIP_APPROX_FAST_CONSTS,
                    RECIPROCAL_APPROX_FAST,
                )
                _c = RECIP_APPROX_FAST_CONSTS
                nc.vector._custom_dve(
                    RECIPROCAL_APPROX_FAST, out=recip[0:NT, :], in0=sums[:],
                    s0=_c["s0"], s1=_c["s1"], imm2=_c["imm2"],
                )
                recipb = psB.tile([NT * NE, BT], f32, tag="bc")
                nc.tensor.matmul(recipb[:], R16sb[:], recip[:], start=True, stop=True)
                gn16 = sbS.tile([NT * NE, BT], f32r, tag="gn16")
                nc.vector.tensor_tensor(gn16[:], expg[0:NT * NE, :], recipb[:], AluOp.mult)
                gn8 = gn8_bufs[i % 2]
                nc.vector.tensor_copy(gn8[0:NT * NE, 0, :], gn16[:])
                nc.gpsimd.tensor_tensor(gn8[0:NT * NE, 1, :], gn16[:],
                                        gn8[0:NT * NE, 0, :], AluOp.subtract)

                # ---- experts: L1 (fp8 DoubleRow) + relu per pair,
                #      L2 (f32r masked pairs) + evac per 2-pair group.
                #      Task-group combine is interleaved right after its evac
                #      so the single psB ring never stalls the PE queue. ----
                def _combine(og, p0, p1, dve_first):
                    gbg = psB.tile([128, 2, BT], f32, tag="bc", name="gbg")
                    nc.tensor.matmul(gbg[:, 0, :], IND8sb[:, p0], gn8[:],
                                     start=True, stop=True, perf_mode=DR)
                    nc.tensor.matmul(gbg[:, 1, :], IND8sb[:, p1], gn8[:],
                                     start=True, stop=True, perf_mode=DR)
                    gated = sbG.tile([128, 2, BT], bf16, tag="gated", name="gated")
                    nc.vector.tensor_tensor(gated[:], og[:], gbg[:], AluOp.mult)
                    a = sbG.tile([128, BT], bf16, tag="acc", name="a")
                    eng = nc.vector if dve_first else nc.gpsimd
                    eng.tensor_tensor(a[:], gated[:, 0, :], gated[:, 1, :],
                                      AluOp.add)
                    return a

                osb_group = {}     # gidx -> SBUF [128, 2, BT] f32r (2 pairs' o)
                task_acc = {}
                for gidx, (ppa, ppb) in enumerate([(6, 7), (0, 1), (2, 3), (4, 5)]):
                    ogrp = psO.tile([128, 2, BT], f32, tag="ogrp")
                    for sl, pp in enumerate((ppa, ppb)):
                        e0, e1 = L2_PAIRS[pp]
                        hgrp = psH.tile([128, 2, BT], f32, tag="h")
                        for he, e in enumerate((e0, e1)):
                            hv = hgrp[:, he, :]
                            nc.tensor.matmul(hv, W18sb[:, e], x8[:],
                                             start=True, stop=False, perf_mode=DR)
                            nc.tensor.matmul(hv, W18sb[:, e], xr8[:],
                                             start=False, stop=False, perf_mode=DR)
                            nc.tensor.matmul(hv, W1R8sb[:, e], x8[:],
                                             start=False, stop=True, perf_mode=DR)
                        hs = sbH.tile([128, 2, BT], f32r, tag="h")
                        if with_bias:
                            for he, e in enumerate((e0, e1)):
                                nc.scalar.activation(hs[:, he, :], hgrp[:, he, :],
                                                     AF.Relu, bias=B1Ssb[:, e:e + 1])
                        else:
                            nc.scalar.activation(hs[:], hgrp[:], AF.Relu)
                        nc.tensor.matmul(ogrp[:, sl, :],
                                         W2sb[:, bass.ts(2 * pp, 128)],
                                         hs[:, 0, :], start=True, stop=False)
                        nc.tensor.matmul(ogrp[:, sl, :],
                                         W2sb[:, bass.ts(2 * pp + 1, 128)],
                                         hs[:, 1, :], start=False, stop=True)
                    og = sbO.tile([128, 2, BT], f32r, tag="osb")
                    if with_bias:
                        for sl, pp in enumerate((ppa, ppb)):
                            nc.scalar.activation(og[:, sl, :], ogrp[:, sl, :],
                                                 AF.Identity,
                                                 bias=B2sb[:, pp:pp + 1],
                                                 scale=1.0 / WS)
                    else:
                        nc.scalar.activation(og[:], ogrp[:], AF.Identity,
                                             scale=1.0 / WS)
                    osb_group[gidx] = og
                    if gidx >= 1:
                        t = gidx - 1
                        task_acc[t] = [_combine(og, t * 4 + 0, t * 4 + 1, True)]

                # shared combines + fold tail
                fold01 = fold2 = None
                for t in range(NT):
                    task_acc[t].append(
                        _combine(osb_group[0], t * 4 + 2, t * 4 + 3, False))
                    at = sbG.tile([128, BT], bf16, tag="acc2")
                    eng = nc.gpsimd if t % 2 == 0 else nc.vector
                    eng.tensor_tensor(at[:], task_acc[t][0][:], task_acc[t][1][:],
                                      AluOp.add)
                    if t == 0:
                        fold01 = psF.tile([128, BT], f32, tag="fold01")
                        nc.tensor.matmul(fold01[0:64, :], FOLDHsb[:], at[:],
                                         start=True, stop=True)
                    elif t == 1:
                        nc.tensor.matmul(fold01[64:128, :], FOLDHsb[:], at[:],
                                         start=True, stop=True)
                    else:
                        fold2 = psF.tile([64, BT], f32, tag="fold2")
                        nc.tensor.matmul(fold2[:], FOLDHsb[:], at[:],
                                         start=True, stop=True)

                # ---- store (bf16) ----
                out01 = sbOut.tile([128, BT], bf16, tag="o01")
                nc.vector.tensor_copy(out01[:], fold01[:])
                out2 = sbOut.tile([64, BT], bf16, tag="o2")
                nc.scalar.activation(out2[:], fold2[:], AF.Identity)
                nc.sync.dma_start(out_dram[0:128, bass.ts(i, BT)], out01[:])
                nc.sync.dma_start(out_dram[128:192, bass.ts(i, BT)], out2[:])

    nc.compile()
    return nc


_NC_CACHE = {}


def _get_nc(with_bias: bool):
    key = f"nc{int(with_bias)}"
    if key not in _NC_CACHE:
        _NC_CACHE[key] = _build_nc(with_bias)
    return _NC_CACHE[key]


def _q8(a):
    return np.asarray(a, F8NP)


def _pack_weights(w1_task, b1_task, w2_task, b2_task, w1_sh, b1_sh, w2_sh, b2_sh, gw, gb):
    # expert order: 12 task-specific (t-major), then 4 shared
    w1_list = [w1_task[t, i] for t in range(NT) for i in range(NS)] + [w1_sh[i] for i in range(NSH)]
    b1_list = [b1_task[t, i] for t in range(NT) for i in range(NS)] + [b1_sh[i] for i in range(NSH)]
    w2_list = [w2_task[t, i] for t in range(NT) for i in range(NS)] + [w2_sh[i] for i in range(NSH)]
    b2_list = [b2_task[t, i] for t in range(NT) for i in range(NS)] + [b2_sh[i] for i in range(NSH)]

    # dual-level fp8 of 32*w1, DoubleRow plane layout [128, NEXP, 2, 128]
    W18 = np.zeros((128, NEXP, 2, 128), F8NP)
    W1R8 = np.zeros((128, NEXP, 2, 128), F8NP)
    for e in range(NEXP):
        ws = WS * w1_list[e]                     # [256, 128]
        hi = _q8(ws)
        lo = _q8(ws - hi.astype(np.float32))
        for k in range(2):
            W18[:, e, k, :] = hi[k * 128:(k + 1) * 128, :]
            W1R8[:, e, k, :] = lo[k * 128:(k + 1) * 128, :]

    W2 = np.zeros((128, NEXP * 128), np.float32)
    for pp, (e0, e1) in enumerate(L2_PAIRS):
        W2[:, (2 * pp) * 128:(2 * pp) * 128 + 64] = w2_list[e0]
        W2[:, (2 * pp + 1) * 128 + 64:(2 * pp + 2) * 128] = w2_list[e1]

    GW8 = np.zeros((128, 2, 32), F8NP)
    GWR8 = np.zeros((128, 2, 32), F8NP)
    for t in range(NT):
        gs = WS * gw[t]                          # [256, 8]
        hi = _q8(gs)
        lo = _q8(gs - hi.astype(np.float32))
        for k in range(2):
            GW8[:, k, t * NE:(t + 1) * NE] = hi[k * 128:(k + 1) * 128, :]
            GWR8[:, k, t * NE:(t + 1) * NE] = lo[k * 128:(k + 1) * 128, :]

    E = np.zeros((128, NT), np.float32)
    for t in range(NT):
        E[t * NE:(t + 1) * NE, t] = 1.0
    R16 = np.zeros((128, NT * NE), np.float32)
    for t in range(NT):
        R16[t, t * NE:(t + 1) * NE] = GS
    IND8 = np.zeros((128, NT * NS, 2, 128), F8NP)
    inv = np.float32(1.0 / GS)
    for t in range(NT):
        for q in range(4):
            p = t * 4 + q
            r0 = t * NE + 2 * q
            r1 = r0 + 1
            for pl in range(2):
                IND8[r0, p, pl, 0:64] = inv
                IND8[r1, p, pl, 64:128] = inv
    FOLDH = np.zeros((128, 64), ml_dtypes.bfloat16)
    for r in range(128):
        FOLDH[r, r % 64] = 1.0

    B1S = WS * np.stack(b1_list, axis=1).astype(np.float32)     # [128, 16]
    B2 = np.empty((128, 8), np.float32)
    for pp, (e0, e1) in enumerate(L2_PAIRS):
        B2[0:64, pp] = b2_list[e0]
        B2[64:128, pp] = b2_list[e1]
    GB = np.empty((NT * NE, 1), np.float32)
    for t in range(NT):
        GB[t * NE:(t + 1) * NE, 0] = gb[t]
    ZPAD = np.zeros((128, BT), np.float32)
    ZPAD8 = np.zeros((128, 2, BT), F8NP)
    return dict(W18=W18, W1R8=W1R8, W2=W2, GW8=GW8, GWR8=GWR8, E=E, R16=R16,
                IND8=IND8, FOLDH=FOLDH, B1S=B1S, B2=B2, GB=GB,
                ZPAD=ZPAD, ZPAD8=ZPAD8)


def kernel(x, w1_task, b1_task, w2_task, b2_task, w1_sh, b1_sh, w2_sh, b2_sh, gw, gb):
    x = np.asarray(x, np.float32)
    b1_task = np.asarray(b1_task, np.float32)
    b2_task = np.asarray(b2_task, np.float32)
    b1_sh = np.asarray(b1_sh, np.float32)
    b2_sh = np.asarray(b2_sh, np.float32)
    weights = _pack_weights(
        np.asarray(w1_task, np.float32), b1_task,
        np.asarray(w2_task, np.float32), b2_task,
        np.asarray(w1_sh, np.float32), b1_sh,
        np.asarray(w2_sh, np.float32), b2_sh,
        np.asarray(gw, np.float32), np.asarray(gb, np.float32),
    )
    with_bias = bool(
        np.any(b1_task) or np.any(b2_task) or np.any(b1_sh) or np.any(b2_sh)
    )
    # dual-level fp8 of x, feature-major DoubleRow planes [128, 2, B]
    x8f = _q8(x)                                  # [B, 256]
    xr8f = _q8(x - x8f.astype(np.float32))
    X8 = np.ascontiguousarray(
        x8f.T.reshape(2, 128, B).transpose(1, 0, 2))     # [128, 2, B]
    XR8 = np.ascontiguousarray(
        xr8f.T.reshape(2, 128, B).transpose(1, 0, 2))

    nc = _get_nc(with_bias)
    in_maps = []
    for c in range(NCORES):
        m = dict(weights)
        m["X8"] = np.ascontiguousarray(X8[:, :, c * BC:(c + 1) * BC])
        m["XR8"] = np.ascontiguousarray(XR8[:, :, c * BC:(c + 1) * BC])
        in_maps.append(m)

    res = run_bass_kernel_spmd(nc, in_maps, list(range(NCORES)))
    _NC_CACHE["last_result"] = res
    if res.exec_time_ns is not None:
        print(f"HW exec time: {res.exec_time_ns} ns")

    outs = []
    for t in range(NT):
        cols = [res.results[c]["out"][t * O:(t + 1) * O, :].astype(np.float32)
                for c in range(NCORES)]
        full = np.concatenate(cols, axis=1)          # [64, B]
        outs.append(np.ascontiguousarray(full.T))    # [B, 64]
    return tuple(outs)
